# revision 61
# baseline (speedup 1.0000x reference)
"""Distributed GQA causal self-attention (RoPE + RMSNorm QK) for 8 TRN2 cores.

Sharding: DP=2 over batch x TP=4 over KV-head groups.
Core c = 4*b + s handles batch b, kv-group s (1 kv head, 4 q heads).
Per-batch ReduceScatter (replica groups [[0..3],[4..7]]) after the output
projection; the host concatenates the 8 scattered shards.

Layout: features on partitions, tokens on the free axis; the only on-device
transpose is tiny (vT -> v):
  qT = WqT.T @ xT          (256, T)   scoresT = knT.T @ qnT   (kt, qt)
  softmax over kt via exp + matmul with ones-augmented v (row 64 = sums)
  yT = v_aug.T @ expT      (65, qt)   outT = WoT.T @ ynT      (1024, T)
RMS-normed q,k bound scores to |s| <= 8, so exp needs no max subtraction.

Schedule: the attention inner loop is a 2-stage software pipeline
(scores/exp for step i issue before the av matmuls of step i-1) and the
projection of chunk n+1 plus the output projection of chunk n-1 are fed
into the loop as PE "filler" quanta, so the tensor engine never idles
behind the exp stream and the scalar engine never idles behind
projection phases.

Head-pairing: the two query heads of a GQA pair share every key tile, so
their score tiles live side by side in one 2-bank PSUM tile [128, 1024] and
a single Exp covers both halves (3D AP on the diagonal) - halving the
activation-engine instruction count.

RMS-norm without act-table thrash: the five sum-of-squares rows land at
engine-legal partition bases across two PSUM tiles (A: rows 0/32/64,
B: rows 0/32), then rrow = sqrt(HD/ss) = Exp(-0.5*Ln(ss) + ln(8)); Ln and
Exp share the `natural_log_exp_and_others` table set, so the scalar engine
never reloads activation tables (the redundant per-switch loads the greedy
compile pass inserts are deduped post-compile into one combined-set load).

Other hardware constraints baked into the schedule: GPSIMD cannot touch
PSUM (all psum->sbuf copies go to ACT/DVE); engine APs need partition
bases in {0,32,64,96}; engine queues are strictly in-order, so any popped
filler quantum must never carry a wait the surrounding attention stream
has not already satisfied.
"""

import numpy as np
from contextlib import ExitStack

B, T, C = 2, 2048, 1024
NH, NKV, HD, HALF = 16, 4, 64, 32
G = NH // NKV          # 4 q heads per kv head
TP, DP = 4, 2
KC = C // 128          # 8 contraction tiles
NT = T // 128          # 16 token tiles
NQ = T // 512          # 4 query chunks
SCALE = 1.0 / np.sqrt(HD)
VS = 72                # column stride of packed v blocks (65 used)
LN8 = float(np.log(8.0))   # 0.5*ln(HD)

_CACHE = {}
SIM_MODE = False


def _dedup_act_table_loads(nc):
    """Replace the greedy per-switch InstLoadActFuncSet stream with a single
    load of the set containing Exp+Ln+Copy. The compile pass runs after
    semaphore generation, so the loads carry no sync state and removal is
    safe; every activation this kernel issues (Exp, Ln, Copy) lives in
    `natural_log_exp_and_others`."""
    import concourse.mybir as mybir
    from concourse.hw_specs import get_activation_tables

    AF = mybir.ActivationFunctionType
    tables = list(get_activation_tables(nc.m.arch).items())
    need = {AF.Exp, AF.Ln, AF.Copy, AF.Identity}
    combined = next(i for i, (name, funcs) in enumerate(tables)
                    if need <= funcs)
    for b in nc.m.functions[0].blocks:
        loads = [i for i in b.instructions
                 if isinstance(i, mybir.InstLoadActFuncSet)]
        for fn_i in b.instructions:
            if isinstance(fn_i, mybir.InstActivation):
                assert fn_i.func in tables[combined][1], fn_i.func
        if loads:
            loads[0].act_func_set_id = combined
            for extra in loads[1:]:
                b.instructions.remove(extra)


def _build():
    import concourse.bass as bass
    import concourse.bacc as bacc
    import concourse.mybir as mybir
    import concourse.tile as tile

    f32 = mybir.dt.float32
    bf16 = mybir.dt.bfloat16
    AF = mybir.ActivationFunctionType

    nc = bacc.Bacc("TRN2", target_bir_lowering=False, debug=False,
                   num_devices=8)

    xT = nc.dram_tensor("xT", [C, T], bf16, kind="ExternalInput").ap()
    cosT = nc.dram_tensor("cosT", [HALF, T], bf16, kind="ExternalInput").ap()
    sinT = nc.dram_tensor("sinT", [HALF, T], bf16, kind="ExternalInput").ap()
    wqT = nc.dram_tensor("wqT", [C, G * HD], bf16, kind="ExternalInput").ap()
    # pre-permuted on host to [128, KC*128] so the DMA runs at 2KB/desc
    wkvT = nc.dram_tensor("wkvT", [128, KC * 128], bf16,
                          kind="ExternalInput").ap()
    woT = nc.dram_tensor("woT", [G * HD, C], bf16, kind="ExternalInput").ap()
    masks = nc.dram_tensor("masks", [128, 128], bf16,
                           kind="ExternalInput").ap()
    eye64 = nc.dram_tensor("eye64", [64, 64], bf16, kind="ExternalInput").ap()
    # bf16 output staging end-to-end (osb -> ar_in -> RS -> outT): halves
    # every epilogue DMA and the collective traffic; the host upcasts the
    # returned array to f32 (costs ~0.4% relative error, gate is 2e-2)
    outT = nc.dram_tensor("outT", [C // 4, T], bf16,
                          kind="ExternalOutput").ap()

    with tile.TileContext(nc) as tc, ExitStack() as es:
        const = es.enter_context(tc.tile_pool(name="const", bufs=1))
        actp = es.enter_context(tc.tile_pool(name="acts", bufs=1))
        dram = es.enter_context(tc.tile_pool(name="dram", bufs=1, space="DRAM"))

        wq_all = const.tile([128, KC * 256], bf16, name="wq", tag="wq")
        wkv_all = const.tile([128, KC * 128], bf16, name="wkv", tag="wkv")
        wo_all = const.tile([128, 2 * C], bf16, name="wo", tag="wo")
        cos_sb = const.tile([128, T], bf16, name="cos", tag="cos")
        sin_sb = const.tile([128, T], bf16, name="sin", tag="sin")
        mask_sb = const.tile([128, 128], bf16, name="mask", tag="mask")
        eye_sb = const.tile([128, 64], bf16, name="eye", tag="eye")

        def wq_dma(q0, nk):
            nc.sync.dma_start(
                wq_all[:, q0 * 256:(q0 + nk) * 256]
                .rearrange("p (k m) -> p k m", m=256),
                wqT[q0 * 128:(q0 + nk) * 128, :]
                .rearrange("(k p) m -> p k m", p=128))

        def table_dmas():
            # cos/sin arrive as [32, T] and are stacked x4 on-device (rope
            # reads them at partition bases 0/32/64/96); two cheap DVE
            # copies replace 3/4 of the HBM traffic on the startup path.
            nc.sync.dma_start(cos_sb[0:HALF, :], cosT)
            nc.sync.dma_start(sin_sb[0:HALF, :], sinT)
            for t in (cos_sb, sin_sb):
                nc.vector.tensor_copy(t[HALF:HD, :], t[0:HALF, :])
                nc.vector.tensor_copy(t[HD:128, :], t[0:HD, :])

        # head-pair selectors for the sum-of-squares matmuls (bf16!)
        onesel = const.tile([128, 2], bf16, name="onesel", tag="onesel")
        nc.any.memset(onesel[:], 0.0)
        nc.any.memset(onesel[0:HALF, 0:1], 1.0)
        nc.any.memset(onesel[HD:HD + HALF, 0:1], 1.0)
        nc.any.memset(onesel[HALF:HD, 1:2], 1.0)
        nc.any.memset(onesel[HD + HALF:128, 1:2], 1.0)
        ones64 = const.tile([64, 1], bf16, name="ones64", tag="ones64")
        nc.any.memset(ones64[:], 1.0)
        ln8c = const.tile([128, 1], f32, name="ln8c", tag="ln8c")
        nc.any.memset(ln8c[:], LN8)

        # ---- persistent activations ----
        q_raw = [actp.tile([128, T], bf16, name=f"qraw{m}", tag=f"qraw{m}")
                 for m in range(2)]
        qnT = [actp.tile([HD, T], bf16, name=f"qn{h}", tag=f"qn{h}")
               for h in range(G)]
        knT = actp.tile([HD, T], bf16, name="kn", tag="kn")
        v_all = actp.tile([128, NT * VS], bf16, name="vall", tag="vall")
        nc.any.memset(v_all[:, HD::VS], 1.0)   # ones column of each v block
        ynT = [actp.tile([128, T], bf16, name=f"yn{m}", tag=f"yn{m}")
               for m in range(2)]

        with tc.tile_pool(name="xTp", bufs=2) as xpool, \
             tc.tile_pool(name="kvp", bufs=3) as kvpool, \
             tc.tile_pool(name="rp", bufs=3) as rp, \
             tc.tile_pool(name="nrm", bufs=3) as nrm, \
             tc.tile_pool(name="expp", bufs=8) as expp, \
             tc.tile_pool(name="smx", bufs=3) as smx, \
             tc.tile_pool(name="psw", bufs=2, space="PSUM") as psw, \
             tc.tile_pool(name="pss", bufs=2, space="PSUM") as pss, \
             tc.tile_pool(name="psy", bufs=2, space="PSUM") as psy:

            def rope6(src, W, c0, c1, rT, tP, t0=None, t1=None):
                """rT[0:W]=x1*cos+x2*sin ; rT[W:2W]=x2*cos-x1*sin."""
                if t0 is None:
                    t0, t1 = c0, c1
                x1 = src[0:W, c0:c1]
                x2 = src[W:2 * W, c0:c1]
                cs, sn = cos_sb[0:W, t0:t1], sin_sb[0:W, t0:t1]
                cs2, sn2 = cos_sb[W:2 * W, t0:t1], sin_sb[W:2 * W, t0:t1]
                nc.vector.tensor_mul(rT[0:W, :], x1, cs)
                nc.vector.tensor_mul(tP[0:W, :], x2, sn2)
                nc.vector.tensor_add(rT[0:W, :], rT[0:W, :], tP[0:W, :])
                nc.vector.tensor_mul(rT[W:2 * W, :], x2, cs2)
                nc.vector.tensor_mul(tP[W:2 * W, :], x1, sn)
                nc.vector.tensor_sub(rT[W:2 * W, :], rT[W:2 * W, :],
                                     tP[W:2 * W, :])

            def rope_q(n, m, ctx):
                """RoPE + x^2 for q m-tile; stashes (rT, sq) in ctx."""
                c0, c1 = n * 512, (n + 1) * 512
                rT = rp.tile([128, 512], bf16, name=f"rq{m}", tag=f"rq{m}")
                tP = rp.tile([128, 512], bf16, name=f"tq{m}", tag=f"tq{m}")
                rope6(q_raw[m], HD, c0, c1, rT, tP)
                s = rp.tile([128, 512], bf16, name=f"sq{m}", tag=f"sq{m}")
                nc.vector.tensor_mul(s[:], rT[:], rT[:])
                ctx["sq"].append((rT, s))

            def rope_k(n, kvr, ctx):
                c0, c1 = n * 512, (n + 1) * 512
                rTk = rp.tile([64, 512], bf16, name="rk", tag="rk")
                tPk = rp.tile([64, 512], bf16, name="tk", tag="tk")
                rope6(kvr, HALF, 0, 512, rTk, tPk, t0=c0, t1=c1)
                sk = rp.tile([64, 512], bf16, name="sk", tag="sk")
                nc.vector.tensor_mul(sk[:], rTk[:], rTk[:])
                ctx["rk"] = rTk
                ctx["sk"] = sk

            def ssq_part(ctx, part):
                """Sum-of-squares matmuls + rrow = Exp(-0.5*Ln(ss)+ln(8)),
                emitted per part (m0 / m1 / k / merged qk) so chunk 0 can
                normalize each piece as soon as its rope lands.

                Engine APs need partition bases in {0,32,64,96}, so the 5
                rms rows live at (A,0)=h0 (A,32)=h1 (A,64)=h2 (B,0)=h3
                (B,32)=k across two psum tiles."""
                if "ssqA" not in ctx:
                    ctx["ssqA"] = psw.tile([65, 512], f32, name="ssqA",
                                           tag="w")
                    ctx["ssqB"] = psw.tile([33, 512], f32, name="ssqB",
                                           tag="w")
                    ctx["rrowA"] = nrm.tile([65, 512], bf16, name="rrowA",
                                            tag="rrowA")
                    ctx["rrowB"] = nrm.tile([33, 512], bf16, name="rrowB",
                                            tag="rrowB")
                    ctx["lnsA"] = nrm.tile([65, 512], f32, name="lnsA",
                                           tag="lnsA")
                    ctx["lnsB"] = nrm.tile([33, 512], f32, name="lnsB",
                                           tag="lnsB")
                A, B = ctx["ssqA"], ctx["ssqB"]
                spans = []

                def mm(dst, d0, d1, sel, rhs, key):
                    first = not ctx.get(key)
                    ctx[key] = True
                    nc.tensor.matmul(dst[d0:d1, :], lhsT=sel, rhs=rhs,
                                     start=True, stop=True,
                                     skip_group_check=not first)

                if part in ("m0", "qk"):
                    mm(A, 0, 1, onesel[:, 0:1], ctx["sq"][0][1][:], "fA")
                    mm(A, 32, 33, onesel[:, 1:2], ctx["sq"][0][1][:], "fA")
                    spans.append(("A", 0, 33))
                if part in ("m1", "qk"):
                    mm(A, 64, 65, onesel[:, 0:1], ctx["sq"][1][1][:], "fA")
                    mm(B, 0, 1, onesel[:, 1:2], ctx["sq"][1][1][:], "fB")
                    spans.append(("A", 64, 65))
                    spans.append(("B", 0, 1))
                if part in ("k", "qk"):
                    mm(B, 32, 33, ones64[:], ctx["sk"][:], "fB")
                    spans.append(("B", 32, 33))
                if part == "qk":
                    spans = [("A", 0, 65), ("B", 0, 33)]
                for t, a, b in spans:
                    ssq = A if t == "A" else B
                    lns = ctx["lnsA" if t == "A" else "lnsB"]
                    rrow = ctx["rrowA" if t == "A" else "rrowB"]
                    nc.scalar.activation(lns[a:b, :], ssq[a:b, :], AF.Ln)
                    nc.scalar.activation(rrow[a:b, :], lns[a:b, :],
                                         AF.Exp, bias=ln8c[0:b - a, :],
                                         scale=-0.5)

            def _bcast(ctx, src_key, rb, h, rows):
                """Stage (if off-base-0) + partition-broadcast one rms row."""
                rr = ctx[src_key]
                if rb > 0:
                    rj = nrm.tile([1, 512], bf16, name=f"rj{h}",
                                  tag=f"rj{h}")
                    nc.vector.tensor_copy(rj[:], rr[rb:rb + 1, :])
                    rr, rb = rj, 0
                bc = nrm.tile([rows, 512], bf16, name=f"bc{h}",
                              tag=f"bc{h}")
                nc.gpsimd.partition_broadcast(bc[:], rr[rb:rb + 1, :])
                return bc

            RSRC = [("rrowA", 0), ("rrowA", 32), ("rrowA", 64),
                    ("rrowB", 0)]

            def norm_m(n, ctx, m):
                """Normalize q heads 2m, 2m+1."""
                c0, c1 = n * 512, (n + 1) * 512
                rT = ctx["sq"][m][0]
                for j in range(2):
                    h = 2 * m + j
                    src, rb = RSRC[h]
                    bc = _bcast(ctx, src, rb, h, 128)
                    r0 = HALF * j
                    nc.vector.tensor_mul(
                        qnT[h][0:HALF, c0:c1], rT[r0:r0 + HALF, :],
                        bc[r0:r0 + HALF, :])
                    nc.vector.tensor_mul(
                        qnT[h][HALF:HD, c0:c1],
                        rT[64 + r0:64 + r0 + HALF, :],
                        bc[64 + r0:64 + r0 + HALF, :])

            def norm_k(n, ctx):
                c0, c1 = n * 512, (n + 1) * 512
                bck = _bcast(ctx, "rrowB", 32, "k", 64)
                nc.vector.tensor_mul(knT[0:HD, c0:c1], ctx["rk"][:],
                                     bck[:])

            def proj_q_mtile(n, xt, m, ctx, fast=False, rope=True):
                """Generator: q projection m-tile, 2 matmuls per quantum."""
                c0, c1 = n * 512, (n + 1) * 512
                qp = psw.tile([128, 512], f32, name="pq", tag="w")
                for k in range(KC):
                    nc.tensor.matmul(
                        qp[:],
                        lhsT=wq_all[:, k * 256 + m * 128:
                                    k * 256 + (m + 1) * 128],
                        rhs=xt[:, k * 512:(k + 1) * 512],
                        start=(k == 0), stop=(k == KC - 1))
                    if k % 2 == 1 and k < KC - 1:
                        yield
                nc.scalar.copy(q_raw[m][:, c0:c1], qp[:])
                if rope:
                    rope_q(n, m, ctx)
                yield

            def proj_kv(n, xt, ctx, fast=False, rope=True):
                c0, c1 = n * 512, (n + 1) * 512
                kvp = psw.tile([128, 512], f32, name="pkv", tag="w")
                for k in range(KC):
                    nc.tensor.matmul(
                        kvp[:], lhsT=wkv_all[:, k * 128:(k + 1) * 128],
                        rhs=xt[:, k * 512:(k + 1) * 512],
                        start=(k == 0), stop=(k == KC - 1))
                    if k % 2 == 1 and k < KC - 1:
                        yield
                kvr = kvpool.tile([128, 512], bf16, name="kvr", tag="kvr")
                nc.scalar.copy(kvr[:], kvp[:])
                if rope:
                    rope_k(n, kvr, ctx)
                ctx["kvr"] = kvr
                yield

            def v_transpose(n, ctx):
                kvr = ctx["kvr"]
                tp = psw.tile([128, 4 * HD], bf16, name="tp", tag="w")
                for it in range(4):
                    nc.tensor.matmul(
                        tp[:, it * HD:(it + 1) * HD],
                        lhsT=kvr[HD:128, it * 128:(it + 1) * 128],
                        rhs=eye_sb[64:128, :], is_transpose=True,
                        skip_group_check=(it > 0))
                nc.vector.tensor_copy(
                    v_all[:].rearrange("p (i s) -> p i s", s=VS)
                    [:, 4 * n:4 * n + 4, 0:HD],
                    tp[:].rearrange("p (i c) -> p i c", c=HD))
                yield

            def proj_gen(n, xt, ctx):
                yield from proj_q_mtile(n, xt, 0, ctx)
                yield from proj_q_mtile(n, xt, 1, ctx)
                yield from proj_kv(n, xt, ctx)
                yield from v_transpose(n, ctx)
                ssq_part(ctx, "qk")
                yield

            def outproj_gen(n, tail=False):
                import concourse.mybir as mybir
                c0, c1 = n * 512, (n + 1) * 512
                ar_in = dram.tile([C, 512], bf16, name=f"arin{n}",
                                  tag=f"arin{n}")
                ar_out = dram.tile([C // 4, 512], bf16, name=f"arout{n}",
                                   tag=f"arout{n}")
                osb = smx.tile([128, 8 * 512], bf16, name="osb", tag="osb",
                               bufs=2)
                for m8 in range(8):
                    op = psw.tile([128, 512], f32, name="o", tag="w")
                    for k2 in range(2):
                        nc.tensor.matmul(
                            op[:],
                            lhsT=wo_all[:, k2 * C + m8 * 128:
                                        k2 * C + (m8 + 1) * 128],
                            rhs=ynT[k2][:, c0:c1],
                            start=(k2 == 0), stop=(k2 == 1))
                        if k2 == 0:
                            yield
                    dst = osb[:, m8 * 512:(m8 + 1) * 512]
                    if tail:
                        # tail-critical: fastest engines only
                        if m8 % 2 == 0:
                            nc.scalar.copy(dst, op[:])
                        else:
                            nc.vector.tensor_copy(dst, op[:])
                    else:
                        # GPSIMD cannot read PSUM, and an ACT copy here
                        # would head-of-line block the exp stream these
                        # quanta are popped into - DVE only.
                        nc.vector.tensor_copy(dst, op[:])
                    nc.sync.dma_start(ar_in[m8 * 128:(m8 + 1) * 128, :],
                                      osb[:, m8 * 512:(m8 + 1) * 512])
                    yield
                if SIM_MODE:
                    # halves: each issues as soon as its ar_in rows land
                    nc.sync.dma_start(outT[0:128, c0:c1], ar_in[0:128, :])
                    nc.sync.dma_start(outT[128:256, c0:c1],
                                      ar_in[128:256, :])
                else:
                    nc.gpsimd.collective_compute(
                        "ReduceScatter", mybir.AluOpType.add,
                        replica_groups=rg,
                        ins=[ar_in.opt()], outs=[ar_out.opt()])
                    nc.sync.dma_start(outT[:, c0:c1], ar_out[:])
                yield

            rg = [[0, 1, 2, 3], [4, 5, 6, 7]]

            def _av_factory(n, yps):
                nkt = 4 * n + 4

                def _av(prev):
                    if not yps:
                        yps.extend(psy.tile([HD + 1, 512], f32,
                                            name=f"y{j}", tag="y")
                                   for j in range(2))
                    i, q0, ex = prev
                    va = v_all[:, i * VS:i * VS + HD + 1]
                    nc.tensor.matmul(
                        yps[0][:, q0:512], lhsT=va, rhs=ex[:, q0:512],
                        start=(i == 0), stop=(i == nkt - 1))
                    nc.tensor.matmul(
                        yps[1][:, q0:512], lhsT=va,
                        rhs=ex[:, 512 + q0:1024],
                        start=(i == 0), stop=(i == nkt - 1))
                return _av

            def attention_pair(n, pr, filler, skip_pops=0):
                """Heads 2*pr, 2*pr+1 share each key tile; one Exp covers
                both score halves. The av matmuls run one step behind the
                scores/exp so PE never blocks on the current exp; filler
                quanta (proj/outproj of neighbor chunks) slot in between.
                skip_pops delays filler pops when the filler's head quanta
                are gated on state this chunk's start hasn't produced yet
                (the popped wait would head-of-line block the PE queue)."""
                c0, c1 = n * 512, (n + 1) * 512
                hA, hB = 2 * pr, 2 * pr + 1
                nkt = 4 * n + 4
                # The first av of a pair carries the psum-ring reuse wait on
                # the PREVIOUS pair's rec/broadcast/ynT normalize chain and
                # would head-of-line block the in-order PE queue; issuing
                # the avs AV_LAG steps behind the scores hides that chain
                # under this pair's first score/exp steps.
                yps = []
                _av = _av_factory(n, yps)
                lag = min(5, nkt - 1)
                pend_av = []
                for i in range(nkt):
                    d = i - 4 * n
                    q0 = 128 * d if d > 0 else 0
                    sp = pss.tile([128, 1024], f32, name="s", tag="s")
                    kt = knT[:, i * 128:(i + 1) * 128]
                    nc.tensor.matmul(
                        sp[:, q0:512], lhsT=kt,
                        rhs=qnT[hA][:, c0 + q0:c1],
                        start=True, stop=True)
                    nc.tensor.matmul(
                        sp[:, 512 + q0:1024], lhsT=kt,
                        rhs=qnT[hB][:, c0 + q0:c1],
                        start=True, stop=True, skip_group_check=True)
                    ex = expp.tile([128, 1024], bf16, name="e", tag="e")
                    if q0:
                        nc.scalar.activation(
                            ex[:].rearrange("p (h q) -> p h q", h=2)
                            [:, :, q0:512],
                            sp[:].rearrange("p (h q) -> p h q", h=2)
                            [:, :, q0:512],
                            AF.Exp, scale=SCALE)
                    else:
                        nc.scalar.activation(ex[:], sp[:], AF.Exp,
                                             scale=SCALE)
                    if d >= 0:
                        meng = nc.vector if n >= 2 else nc.gpsimd
                        meng.tensor_mul(
                            ex[:, 128 * d:128 * (d + 1)],
                            ex[:, 128 * d:128 * (d + 1)], mask_sb[:])
                        meng.tensor_mul(
                            ex[:, 512 + 128 * d:512 + 128 * (d + 1)],
                            ex[:, 512 + 128 * d:512 + 128 * (d + 1)],
                            mask_sb[:])
                    # no pops on diagonal steps (except tiny chunk 0): a
                    # popped osb DVE copy would delay the diag mask muls
                    # and the trailing avs; the reserved quanta drain at
                    # the pair boundary instead, where PE idles anyway
                    if i >= skip_pops and (d < 0 or n == 0):
                        filler.pop()
                    pend_av.append((i, q0, ex))
                    if len(pend_av) > lag:
                        _av(pend_av.pop(0))
                while pend_av:
                    _av(pend_av.pop(0))
                for j, h in ((0, hA), (1, hB)):
                    yp = yps[j]
                    rec = smx.tile([1, 512], f32, name="rec", tag="rec")
                    nc.vector.reciprocal(rec[:], yp[HD:HD + 1, :])
                    bc2 = smx.tile([HD, 512], f32, name="bc2", tag="bc2")
                    nc.gpsimd.partition_broadcast(bc2[:], rec[:])
                    m_, r0 = h // 2, 64 * (h % 2)
                    nc.vector.tensor_mul(ynT[m_][r0:r0 + HD, c0:c1],
                                         yp[0:HD, :], bc2[:])

            def xt_dma(n, parts=1):
                c0, c1 = n * 512, (n + 1) * 512
                xt = xpool.tile([128, KC * 512], bf16, name="xt", tag="xt")
                kq = KC // parts
                for h in range(parts):
                    nc.sync.dma_start(
                        xt[:, h * kq * 512:(h + 1) * kq * 512]
                        .rearrange("p (k t) -> p k t", t=512),
                        xT[h * kq * 128:(h + 1) * kq * 128, c0:c1]
                        .rearrange("(k p) t -> p k t", p=128))
                    if n == 0 and h == 0:
                        wq_dma(0, 4)
                        table_dmas()
                    elif n == 0 and h == 1:
                        nc.sync.dma_start(wkv_all[:], wkvT)
                return xt

            def drain(g):
                for _ in g:
                    pass

            class Filler:
                """In-order queue of PE-quantum generators; proj work (first
                gen) drains before deferred outproj work."""

                def __init__(self, gens):
                    self.gens = [iter(g) for g in gens]

                def pop(self, k=1):
                    for _ in range(k):
                        while self.gens:
                            if next(self.gens[0], _S) is _S:
                                self.gens.pop(0)
                            else:
                                break
                        if not self.gens:
                            return

                def drain_gen(self, idx=0):
                    if idx < len(self.gens):
                        for _ in self.gens[idx]:
                            pass

                def drain(self):
                    while self.gens:
                        self.pop()

            _S = object()

            # ---------------- prologue: chunk 0 ----------------
            # DMA order: first q-proj gates (xt quarters + wq halves),
            # then rope tables, then kv / aux, then chunk-1 x.
            xt0 = xt_dma(0, parts=4)
            wq_dma(4, 4)
            nc.sync.dma_start(eye_sb[64:128, :], eye64)
            nc.sync.dma_start(mask_sb[:], masks)
            ctx0 = {"sq": []}
            ctxs = {0: ctx0}
            # Dense PE order (m0, kv, m1, vt) keeps the p-state ramp warm
            # and copies go to the idle ACT engine.  RoPE/rms/normalize are
            # emitted per piece in DVE dependency order (m0 -> k -> m1) so
            # the first attention pair's inputs (heads 0,1 + k) are ready
            # while m1 is still normalizing.
            drain(proj_q_mtile(0, xt0, 0, ctx0, fast=True, rope=True))
            drain(proj_kv(0, xt0, ctx0, fast=True, rope=False))
            drain(proj_q_mtile(0, xt0, 1, ctx0, fast=True, rope=False))
            drain(v_transpose(0, ctx0))
            ssq_part(ctx0, "m0")
            norm_m(0, ctx0, 0)
            rope_k(0, ctx0["kvr"], ctx0)
            ssq_part(ctx0, "k")
            norm_k(0, ctx0)
            xt_next = xt_dma(1, parts=2)
            nc.sync.dma_start(
                wo_all[:].rearrange("p (k m) -> p k m", m=C),
                woT.rearrange("(k p) m -> p k m", p=128))
            ctxs[1] = {"sq": []}
            pg1 = proj_gen(1, xt_next, ctxs[1])
            for _ in range(3):
                next(pg1, None)   # chunk-1 m0 projection fills the rms gap

            def ssq_m1_gen(ctx):
                """Chunk-0 m1 rms piece, drained between pair0 and pair1 so
                its PE matmul never head-of-line blocks pair0."""
                rope_q(0, 1, ctx)
                ssq_part(ctx, "m1")
                norm_m(0, ctx, 1)
                yield

            # ---------------- main loop ----------------
            # Per chunk: pair0, [norm n+1 mid-chunk], pair1; outproj(n) is
            # NOT emitted at its own boundary - it becomes the next chunk's
            # pair0 filler, so its wait on ynT[1](n) (the pair1 softmax-
            # normalize chain) resolves while pair0(n+1) already streams.
            pend = []              # deferred outproj generators
            for n in range(NQ):
                if n + 2 < NQ:
                    xt_n2 = xt_dma(n + 2)
                gens = []
                if n == 0:
                    gens = [pg1]
                elif n + 1 < NQ:
                    ctxs[n + 1] = {"sq": []}
                    gens = [proj_gen(n + 1, xt_next, ctxs[n + 1])]
                fill = Filler(gens + pend)
                pend = []
                attention_pair(n, 0, fill,
                               skip_pops=3 if n == NQ - 1 else 0)
                if n == 0:
                    drain(ssq_m1_gen(ctx0))
                if n + 1 < NQ:
                    fill.drain_gen(0)   # proj must finish before norm
                    norm_m(n + 1, ctxs[n + 1], 0)
                    norm_m(n + 1, ctxs[n + 1], 1)
                    norm_k(n + 1, ctxs[n + 1])
                    xt_next = xt_n2 if n + 2 < NQ else None
                attention_pair(n, 1, fill)
                if n == NQ - 1:
                    fill.drain()
                    drain(outproj_gen(n, tail=True))
                else:
                    pend = list(fill.gens) + [outproj_gen(n)]

    nc.compile()
    _dedup_act_table_loads(nc)
    return nc


def _get_nc():
    if "nc" not in _CACHE:
        _CACHE["nc"] = _build()
    return _CACHE["nc"]


def _make_masks():
    p = np.arange(128)[:, None]
    c = np.arange(128)[None, :]
    return (c >= p).astype(np.float32)


def _bf16(a):
    import ml_dtypes
    return np.ascontiguousarray(np.asarray(a).astype(ml_dtypes.bfloat16))


def kernel(x, cos, sin, Wq, Wk, Wv, Wo, _trace=False):
    from concourse.bass_utils import run_bass_kernel_spmd

    nc = _get_nc()
    cosT = _bf16(np.asarray(cos)[0, :, 0, :].T.astype(np.float32))
    sinT = _bf16(np.asarray(sin)[0, :, 0, :].T.astype(np.float32))
    # head-pair permutation of q columns within each 128-wide m-tile:
    # [hA.x1 | hB.x1 | hA.x2 | hB.x2]
    perm = np.zeros(256, dtype=np.int64)
    for mm in range(2):
        base = 128 * mm
        hA, hB = 128 * mm, 128 * mm + 64
        perm[base:base + 32] = hA + np.arange(32)
        perm[base + 32:base + 64] = hB + np.arange(32)
        perm[base + 64:base + 96] = hA + 32 + np.arange(32)
        perm[base + 96:base + 128] = hB + 32 + np.arange(32)
    masks = _bf16(_make_masks())
    eye = _bf16(np.eye(64, dtype=np.float32))
    in_maps = []
    for b in range(DP):
        xT = _bf16(np.asarray(x)[b].T)
        for s in range(TP):
            wkv = np.concatenate([np.asarray(Wk)[64 * s:64 * (s + 1), :],
                                  np.asarray(Wv)[64 * s:64 * (s + 1), :]],
                                 axis=0)
            wkvT_pre = wkv.T.reshape(KC, 128, 2 * HD).transpose(
                1, 0, 2).reshape(128, KC * 2 * HD)
            in_maps.append({
                "xT": xT,
                "cosT": cosT,
                "sinT": sinT,
                "wqT": _bf16(np.asarray(Wq)[256 * s:256 * (s + 1), :]
                             .T[:, perm]),
                "wkvT": _bf16(wkvT_pre),
                "woT": _bf16(np.asarray(Wo)[:, 256 * s:256 * (s + 1)].T),
                "masks": masks,
                "eye64": eye,
            })
    res = run_bass_kernel_spmd(nc, in_maps, core_ids=list(range(8)),
                               trace=_trace)
    out = np.stack([
        np.concatenate([res.results[c]["outT"] for c in range(4)], axis=0).T,
        np.concatenate([res.results[c]["outT"] for c in range(4, 8)],
                       axis=0).T])
    if _trace:
        _CACHE["last_result"] = res
    return np.ascontiguousarray(out, dtype=np.float32)
